# revision 1
# baseline (speedup 1.0000x reference)
"""Banded DTW loss kernel for Trainium2 (Bass/Tile), 8-core data-parallel.

Bidirectional (meet-in-the-middle) formulation, v2:
  The 1024-row banded DP is split into two 512-row halves that run
  SIMULTANEOUSLY in the same [8,41] DVE ops: lanes 0:4 = forward DP of
  rows 0..511 (4 samples), lanes 4:8 = forward DP of the REVERSED
  sequences (= reverse DP of rows 1023..512, mirrored coords).  This
  halves both serial chains (512 DP iterations of 2 ops, 511 walk steps
  of 1 op).  Any cost-optimal DTW path gives the same Sx+Sy (== the DP
  optimum); only the tiny BCE/cnt term depends on tie-breaking, so exact
  reference tie-order is not required.

  Phase A: 512 rows x (min + tensor_tensor_scan) on [8,41]; rolling
           128-slot D window + 256-slot d ring; one plain-slice DMA per
           32-row block stages d in (ring) and D out (RE), interleaved
           with the row loop so nothing stalls.
  Meet:    total(o) = F[511,o] + min(R[512,o-1], R[512,o]); first argmin
           picks the crossing; seeds both walks.
  Phase B: choice bits + g/L scans in an RE layout of 32 blocks
           (partition = 32*sample + (rv-1)%32, block = (rv-1)//32;
           F rows at blocks 0..15, mirrored-R rows at blocks 16..31;
           virtual boundary rows are memset directly into the DrePrev
           copy at (p=0, b=0/16) and survive the shift DMAs).
  Walk:    511 shared one-hot extract steps on [8,41]; per-block g
           tables DMA'd in walk order so the stream stays ahead.
  Masks/metrics: interval masks from (lo, x) per row; 4 aggregates
           reduced per partition; host sums partials.

Sharding: batch 32 -> 4 samples per core on 8 cores; host sums partials.
"""

import numpy as np

import concourse.bacc as bacc
import concourse.bass as bass
import concourse.mybir as mybir
import concourse.tile as tile
from concourse.bass_utils import run_bass_kernel_spmd

B, N, NF = 32, 1024, 4
W = 20
NCORES = 8
BC = B // NCORES          # samples per core
BIG = 1e30
NB = 41                   # band width
CW = 43                   # RE row width (col 0 pad, col c=o+1, col 42 pad)
NBLK = 32                 # RE blocks (F: b 0..15, R': b 16..31)
HBLK = 16
H = N // 2                # 512 DP rows per half
RING = 256                # d ring slots (41 wide)
NWIN = 128                # D window slots (42 wide, col 41 = BIG pad)
SKW2 = 522                # skew array width per half
RSK = 560                 # R'-half skew offset inside R1/R2/R3 regions

AL = mybir.AluOpType
DT = mybir.dt.float32
AX = mybir.AxisListType

# ---- megaQ ([128, QW]) regions; DP lanes live on partitions 0:8 ----
WIN_O = 0                          # 128 slots * 42
VR_O = WIN_O + NWIN * 42           # virtual row (42)
RING_O = VR_O + 42                 # 256 slots * 41
TMP_O = RING_O + RING * NB         # 48
WSC_O = TMP_O + 48                 # walk scratch (48)
XF_O = WSC_O + 48                  # xfull (516)
DUMP_O = XF_O + 516                # spill for p=0 rows of gwalk blocks (41)
GW_O = DUMP_O + NB                 # walk g tables: 511 * 41
MEET_O = GW_O + 511 * NB           # meet scratch
MRR_O = MEET_O                     # 43: Rreal padded (col0 BIG)
MTV_O, MTD_O, MTO_O, MWS_O = (MEET_O + 43 + i * NB for i in range(4))
MSC_O = MWS_O + NB                 # scalars: MN, OS, TVS, TDS, VF, T1
QW = MSC_O + 16

# ---- megaRE ([128, REW]) regions ----
RE = NBLK * CW                     # 1376
R1_O, R2_O, R3_O, R4_O, R5_O, R6_O, R7_O, R8_O = (i * RE for i in range(8))
SM_O = 8 * RE
PX_O, PY_O, PZ_O = SM_O, SM_O + NBLK, SM_O + 2 * NBLK
XC_O, OLO_O = SM_O + 3 * NBLK, SM_O + 4 * NBLK
COLIO_O = SM_O + 5 * NBLK          # 43 values 0..42
DESC_O = COLIO_O + 43              # 41 values 41..1
CLZ_O = DESC_O + 41
SPZ_O, SPN_O, QZ_O = CLZ_O + NBLK, CLZ_O + 2 * NBLK, CLZ_O + 3 * NBLK
RED_O = CLZ_O + 4 * NBLK           # Sx, Sy, Sbce, cnt
REW = RED_O + 8

_CACHE = {}


def _ap(t, part0, off, dims):
    """AP at partition `part0`, col offset `off`, explicit [stride,count]
    dims (strides in elements; partition stride = tile pitch)."""
    base = t[part0:part0 + 1, 0:1]
    return bass.AP(base.tensor, base.offset + off, [list(d) for d in dims])


def _build_module():
    nc = bacc.Bacc("TRN2", target_bir_lowering=False, debug=False,
                   num_devices=NCORES)
    pre = nc.dram_tensor("pre", [128, 3 * NBLK], DT, kind="ExternalInput")
    tsk = nc.dram_tensor("tsk", [128, 6 * SKW2], DT, kind="ExternalInput")
    cst = nc.dram_tensor("cst", [128, 84], DT, kind="ExternalInput")
    partials = nc.dram_tensor("partials", [128, 4], DT, kind="ExternalOutput")

    with tile.TileContext(nc) as tc:
        with tc.tile_pool(name="main", bufs=1) as pool:
            megaQ = pool.tile([128, QW], DT)
            megaRE = pool.tile([128, REW], DT)
            _emit(nc, megaQ, megaRE, pre, tsk, cst, partials)
    nc.compile()
    return nc


def _emit(nc, megaQ, megaRE, pre, tsk, cst, partials):
    v = nc.vector
    QP = QW      # megaQ partition pitch
    RP = REW     # megaRE partition pitch

    def cells(off, dc=0, b0=0, nb=NBLK):
        s = off + b0 * CW
        return megaRE[:, s:s + nb * CW].rearrange(
            "p (b c) -> p b c", c=CW)[:, :, 1 + dc:NB + 1 + dc]

    def smb(off, b0=0, nb=NBLK):
        return megaRE[:, off + b0:off + b0 + nb].unsqueeze(2) \
            .broadcast_to([128, nb, NB])

    def ocolv(shift=0, nb=NBLK):
        s = COLIO_O + 1 + shift
        return megaRE[:, s:s + NB].unsqueeze(1).broadcast_to([128, nb, NB])

    def skwin(off, skb, nb):
        base = megaRE[:, off + skb:off + skb + 1]
        ap0 = [list(base.ap[0])]
        return bass.AP(base.tensor, base.offset,
                       ap0 + [[32, nb], [1, NB]])

    # ---------------- input DMAs ----------------
    # Ordered by first use on the serialized HWDGE: the d build's first
    # op needs the tx skews + pre; cst (iota tables) and the tz skew
    # (BCE cells under phase A) come last.
    nc.sync.dma_start(out=megaRE[:, R1_O:R1_O + SKW2], in_=tsk[:, 0:SKW2])
    nc.sync.dma_start(out=megaRE[:, R1_O + RSK:R1_O + RSK + SKW2],
                      in_=tsk[:, 3 * SKW2:4 * SKW2])
    nc.sync.dma_start(out=megaRE[:, PX_O:PX_O + 3 * NBLK], in_=pre[:])
    nc.sync.dma_start(out=megaRE[:, R2_O:R2_O + SKW2],
                      in_=tsk[:, SKW2:2 * SKW2])
    nc.sync.dma_start(out=megaRE[:, R2_O + RSK:R2_O + RSK + SKW2],
                      in_=tsk[:, 4 * SKW2:5 * SKW2])
    nc.sync.dma_start(out=megaRE[:, COLIO_O:COLIO_O + 84], in_=cst[:])
    nc.sync.dma_start(out=megaRE[:, R3_O:R3_O + SKW2],
                      in_=tsk[:, 2 * SKW2:3 * SKW2])
    nc.sync.dma_start(out=megaRE[:, R3_O + RSK:R3_O + RSK + SKW2],
                      in_=tsk[:, 5 * SKW2:6 * SKW2])

    # ---------------- init memsets ----------------
    v.memset(_ap(megaQ, 0, WIN_O + 41, [[QP, 8], [42, NWIN]]), BIG)  # win pads
    v.memset(megaQ[0:8, VR_O:VR_O + 42], BIG)
    v.memset(megaQ[0:8, VR_O + 20:VR_O + 21], 0.0)       # DP origin (o=20)
    v.memset(megaQ[0:4, MRR_O:MRR_O + 1], BIG)           # meet pad
    v.memset(_ap(megaRE, 0, R4_O, [[RP, 128], [CW, NBLK]]), BIG)      # pads
    v.memset(_ap(megaRE, 0, R4_O + 42, [[RP, 128], [CW, NBLK]]), BIG)
    v.memset(_ap(megaRE, 0, R5_O, [[RP, 128], [CW, NBLK]]), BIG)
    v.memset(_ap(megaRE, 0, R5_O + 42, [[RP, 128], [CW, NBLK]]), BIG)
    v.memset(megaRE[:, XC_O:XC_O + NBLK], 0.0)

    # ---------------- BCE scalar prep (Act engine; runs under phase A) ----
    v.tensor_scalar(out=megaRE[:, CLZ_O:CLZ_O + NBLK],
                    in0=megaRE[:, PZ_O:PZ_O + NBLK],
                    scalar1=-4.0, scalar2=4.0, op0=AL.max, op1=AL.min)
    nc.scalar.activation(megaRE[:, SPN_O:SPN_O + NBLK],
                         megaRE[:, CLZ_O:CLZ_O + NBLK],
                         mybir.ActivationFunctionType.Exp)
    nc.scalar.activation(megaRE[:, SPZ_O:SPZ_O + NBLK],
                         megaRE[:, SPN_O:SPN_O + NBLK],
                         mybir.ActivationFunctionType.Ln, bias=1.0)
    nc.scalar.activation(megaRE[:, QZ_O:QZ_O + NBLK],
                         megaRE[:, CLZ_O:CLZ_O + NBLK],
                         mybir.ActivationFunctionType.Exp, scale=-1.0)
    nc.scalar.activation(megaRE[:, SPN_O:SPN_O + NBLK],
                         megaRE[:, QZ_O:QZ_O + NBLK],
                         mybir.ActivationFunctionType.Ln, bias=1.0)
    v.scalar_tensor_tensor(out=megaRE[:, QZ_O:QZ_O + NBLK],
                           in0=megaRE[:, SPN_O:SPN_O + NBLK], scalar=5.0,
                           in1=megaRE[:, SPZ_O:SPZ_O + NBLK],
                           op0=AL.mult, op1=AL.subtract)

    # ---------------- d build ----------------
    # Band validity needs no explicit mask: the host poisons out-of-range
    # target x/y values with 5e14, so d at invalid cells is ~1e15 (BIG-
    # like for the DP) while valid cells get exact |dx|+|dy|.
    for b0, skb in ((0, 0), (HBLK, RSK)):
        v.tensor_tensor(out=cells(R5_O, 0, b0, HBLK), in0=smb(PX_O, b0, HBLK),
                        in1=skwin(R1_O, skb, HBLK), op=AL.subtract)
    v.scalar_tensor_tensor(out=cells(R1_O), in0=cells(R5_O), scalar=-1.0,
                           in1=cells(R5_O), op0=AL.mult, op1=AL.max)
    for b0, skb in ((0, 0), (HBLK, RSK)):
        v.tensor_tensor(out=cells(R5_O, 0, b0, HBLK), in0=smb(PY_O, b0, HBLK),
                        in1=skwin(R2_O, skb, HBLK), op=AL.subtract)
    v.scalar_tensor_tensor(out=cells(R2_O), in0=cells(R5_O), scalar=-1.0,
                           in1=cells(R5_O), op0=AL.mult, op1=AL.max)
    v.tensor_tensor(out=cells(R6_O), in0=cells(R1_O), in1=cells(R2_O),
                    op=AL.add)                         # dcost -> R6
    # virtual boundary rows for DrePrev: p=0 partitions only (quadrant
    # starts are legal DVE start partitions); nothing else writes them.
    for s in range(4):
        p0 = 32 * s
        for hb in (0, HBLK):
            v.memset(megaRE[p0:p0 + 1,
                            R5_O + hb * CW:R5_O + hb * CW + CW], BIG)
            v.memset(megaRE[p0:p0 + 1,
                            R5_O + hb * CW + 21:R5_O + hb * CW + 22], 0.0)
    v.memset(megaRE[:, RED_O + 1:RED_O + 2], 0.0)      # unused Sy slot

    # ---------------- staging DMAs (per 32-row block) ----------------
    def dstage(b):
        """d for rows rv=32b+1..32b+32 (block b F, b+16 R') -> ring."""
        for hb, lane in ((0, 0), (HBLK, 4)):
            nc.sync.dma_start(
                out=_ap(megaQ, lane, RING_O + ((32 * b) % RING) * NB,
                        [[QP, 4], [NB, 32], [1, NB]]),
                in_=megaRE[:, R6_O + (hb + b) * CW + 1:
                           R6_O + (hb + b) * CW + 1 + NB])

    def evac(b):
        """window slots for rows r=32b+1..32b+32 -> R4 block b / b+16."""
        s0 = WIN_O + ((32 * b) % NWIN) * 42
        for hb, lane in ((0, 0), (HBLK, 4)):
            # SP queue: lower DGE start latency (650 vs 784 ns) on the
            # evac(15) -> choice-fast critical edge
            nc.sync.dma_start(
                out=megaRE[:, R4_O + (hb + b) * CW + 1:
                           R4_O + (hb + b) * CW + 1 + NB],
                in_=_ap(megaQ, lane, s0, [[QP, 4], [42, 32], [1, NB]]))

    for b in range(8):
        dstage(b)

    # ---------------- phase A: 512 rows, 2 DVE ops each ----------------
    tmp8 = megaQ[0:8, TMP_O:TMP_O + NB]
    for r in range(1, H + 1):
        wp = VR_O if r == 1 else WIN_O + ((r - 2) % NWIN) * 42
        ws = WIN_O + ((r - 1) % NWIN) * 42
        rc = RING_O + ((r - 1) % RING) * NB
        v.tensor_tensor(out=tmp8, in0=megaQ[0:8, wp:wp + NB],
                        in1=megaQ[0:8, wp + 1:wp + NB + 1], op=AL.min)
        v.tensor_tensor_scan(out=megaQ[0:8, ws:ws + NB], data0=tmp8,
                             data1=megaQ[0:8, rc:rc + NB], initial=BIG,
                             op0=AL.min, op1=AL.add)
        if r % 32 == 0:
            bdone = r // 32 - 1
            evac(bdone)
            if bdone + 8 < HBLK:
                dstage(bdone + 8)

    # ---------------- meet ----------------
    w511 = WIN_O + 127 * 42
    f511 = megaQ[0:4, w511:w511 + NB]
    nc.sync.dma_start(
        out=megaQ[0:4, MRR_O + 1:MRR_O + 42],
        in_=_ap(megaQ, 4, w511 + 40, [[QP, 4], [-1, NB]]))
    tv = megaQ[0:4, MTV_O:MTV_O + NB]
    td = megaQ[0:4, MTD_O:MTD_O + NB]
    tot = megaQ[0:4, MTO_O:MTO_O + NB]
    mws = megaQ[0:4, MWS_O:MWS_O + NB]
    mn = megaQ[0:4, MSC_O:MSC_O + 1]
    osr = megaQ[0:4, MSC_O + 1:MSC_O + 2]
    tvs = megaQ[0:4, MSC_O + 2:MSC_O + 3]
    tds = megaQ[0:4, MSC_O + 3:MSC_O + 4]
    vf = megaQ[0:4, MSC_O + 4:MSC_O + 5]
    t1 = megaQ[0:4, MSC_O + 5:MSC_O + 6]
    iota0 = megaRE[0:4, COLIO_O:COLIO_O + NB]            # 0..40
    desc = megaRE[0:4, DESC_O:DESC_O + NB]               # 41..1
    v.tensor_tensor(out=tv, in0=f511, in1=megaQ[0:4, MRR_O:MRR_O + NB],
                    op=AL.add)
    v.tensor_tensor(out=td, in0=f511, in1=megaQ[0:4, MRR_O + 1:MRR_O + 42],
                    op=AL.add)
    v.tensor_tensor(out=tot, in0=tv, in1=td, op=AL.min)
    v.tensor_reduce(out=mn, in_=tot, axis=AX.X, op=AL.min)
    v.scalar_tensor_tensor(out=mws, in0=tot, scalar=mn, in1=desc,
                           op0=AL.is_equal, op1=AL.mult)
    v.tensor_reduce(out=osr, in_=mws, axis=AX.X, op=AL.max)
    v.tensor_scalar(out=osr, in0=osr, scalar1=-1.0, scalar2=41.0,
                    op0=AL.mult, op1=AL.add)             # o*
    v.scalar_tensor_tensor(out=mws, in0=iota0, scalar=osr, in1=tv,
                           op0=AL.is_equal, op1=AL.mult, accum_out=tvs)
    # vertical iff tv achieves the min at o* (tot[o*] == mn exactly)
    v.tensor_tensor(out=vf, in0=tvs, in1=mn, op=AL.is_equal)
    v.tensor_scalar(out=megaQ[0:4, XF_O + 511:XF_O + 512], in0=osr,
                    scalar1=1.0, scalar2=0.0, op0=AL.add, op1=AL.add)
    v.tensor_tensor(out=t1, in0=vf, in1=osr, op=AL.subtract)
    v.tensor_scalar(out=t1, in0=t1, scalar1=41.0, scalar2=0.0,
                    op0=AL.add, op1=AL.add)              # 41 - o* + vf
    nc.sync.dma_start(out=_ap(megaQ, 4, XF_O + 511, [[QP, 4], [1, 1]]),
                      in_=_ap(megaQ, 0, MSC_O + 5, [[QP, 4], [1, 1]]))

    # ---------------- BCE cells (mask-independent; fills DrePrev wait) --
    for b0, skb in ((0, 0), (HBLK, RSK)):
        v.tensor_tensor(out=cells(R1_O, 0, b0, HBLK),
                        in0=skwin(R3_O, skb, HBLK),
                        in1=smb(QZ_O, b0, HBLK), op=AL.mult)
        v.tensor_tensor(out=cells(R1_O, 0, b0, HBLK),
                        in0=cells(R1_O, 0, b0, HBLK),
                        in1=smb(SPZ_O, b0, HBLK), op=AL.add)

    # ---------------- phase B: DrePrev, choice bits, g/L scans ----------
    # R5 slot (p,b) = D of rv-1.  Per-sample partition shifts (31-wide, so
    # the p=0 partitions keep the early-memset virtual rows), plus p=0
    # fixups for b>=1 from the previous block's p=31 row.  Everything is
    # chunked by block range, high blocks first, so the walk's dependency
    # chain only runs through blocks 13..15/29..31; Tile streams the rest
    # underneath the walk.
    QS = (nc.scalar, nc.sync, nc.gpsimd)

    def dreprev(blo, bhi):
        w = (bhi - blo + 1) * CW
        for s in range(4):
            p0 = 32 * s
            for i, hb in enumerate((0, HBLK)):
                base = R5_O + (hb + blo) * CW
                src = R4_O + (hb + blo) * CW
                QS[(2 * s + i) % 3].dma_start(
                    out=megaRE[p0 + 1:p0 + 32, base:base + w],
                    in_=megaRE[p0:p0 + 31, src:src + w])
                flo = max(blo, 1) if hb == 0 else max(blo, 1 + HBLK) - HBLK
                if flo <= bhi:
                    fw = (bhi - flo + 1) * CW
                    fb = R5_O + (hb + flo) * CW
                    fs = R4_O + (hb + flo - 1) * CW
                    QS[(2 * s + i + 1) % 3].dma_start(
                        out=megaRE[p0:p0 + 1, fb:fb + fw],
                        in_=megaRE[p0 + 31:p0 + 32, fs:fs + fw])

    def choice(blo, nb):
        # isleft -> R7, isdiag/gval -> R8, notleft -> R2, Lval -> R3
        for b0 in (blo, HBLK + blo):
            diag = cells(R5_O, 0, b0, nb)
            up = cells(R5_O, 1, b0, nb)
            left = cells(R4_O, -1, b0, nb)
            c2 = cells(R2_O, 0, b0, nb)
            c3 = cells(R3_O, 0, b0, nb)
            c7 = cells(R7_O, 0, b0, nb)
            c8 = cells(R8_O, 0, b0, nb)
            v.tensor_tensor(out=c2, in0=diag, in1=up, op=AL.min)
            v.tensor_tensor(out=c7, in0=left, in1=c2, op=AL.is_lt)  # isleft
            v.tensor_tensor(out=c2, in0=left, in1=up, op=AL.min)
            v.tensor_tensor(out=c8, in0=diag, in1=c2, op=AL.is_le)  # isdiag
            v.tensor_single_scalar(out=c2, in_=c7, scalar=0.0,
                                   op=AL.is_equal)                  # notleft
            v.tensor_tensor(out=c3, in0=ocolv(1, nb), in1=c2, op=AL.mult)
            v.tensor_tensor(out=c8, in0=c3, in1=c8, op=AL.subtract)  # gval
            v.tensor_tensor(out=c3, in0=c3, in1=c2, op=AL.subtract)  # Lval

    # Fast path for blocks 13..15/29..31: their DrePrev rows (D rows
    # 416..511) are still live in the window ring (slots 31..126), which
    # is row-linear, so one plain DMA per (block, half) covers all p
    # including p=0 — and it does not wait on the evac hop.
    for i, b in enumerate((15, 14, 13)):
        s0 = WIN_O + ((32 * b - 1) % NWIN) * 42
        for hb, lane in ((0, 0), (HBLK, 4)):
            QS[(2 * i + (hb > 0)) % 3].dma_start(
                out=megaRE[:, R5_O + (hb + b) * CW + 1:
                           R5_O + (hb + b) * CW + 1 + NB],
                in_=_ap(megaQ, lane, s0, [[QP, 4], [42, 32], [1, NB]]))
    choice(13, 3)
    dreprev(7, 12)
    dreprev(0, 6)
    choice(7, 6)
    choice(0, 7)
    # g scans + gwalk DMAs in DESCENDING block order so the walk (which
    # consumes gw slots high-k first) can start as soon as pair 15/31 is
    # staged; the rest streams underneath it.  gw slot col (k-1)*41 holds
    # g of row k: F rv=k+1 at (p=k%32, b=k//32), R' at b=16+k//32; the
    # unused p=0 row of block 0 lands in the DUMP spill region.
    for b in range(HBLK - 1, -1, -1):
        for hb in (0, HBLK):
            v.tensor_tensor_scan(
                out=megaRE[:, R5_O + (hb + b) * CW + 1:
                           R5_O + (hb + b) * CW + 1 + NB],
                data0=megaRE[:, R7_O + (hb + b) * CW + 1:
                             R7_O + (hb + b) * CW + 1 + NB],
                data1=megaRE[:, R8_O + (hb + b) * CW + 1:
                             R8_O + (hb + b) * CW + 1 + NB],
                initial=0.0, op0=AL.mult, op1=AL.add)    # gfull -> R5
        for hb, lane in ((0, 0), (HBLK, 4)):
            nc.gpsimd.dma_start(
                out=_ap(megaQ, lane, GW_O + (32 * b - 1) * NB,
                        [[QP, 4], [NB, 32], [1, NB]]),
                in_=megaRE[:, R5_O + (hb + b) * CW + 1:
                           R5_O + (hb + b) * CW + 1 + NB])

    # ---------------- per-chunk mask + metric cells -------------------
    # Everything except three final reductions is computed per 4-block
    # chunk as soon as the walk has produced that chunk's xcol columns,
    # so it streams into the walk's dependency-latency gaps.
    # ---------------- walk: 511 shared steps + streamed extras ---------
    # xcol block b (xfull cols 32b..32b+31 -> XC col b / b+16) is emitted
    # as soon as walk step k=32b+1 has produced col 32b; tail chunk j
    # follows once its 4 blocks' xcols are all emitted.
    iot8 = megaRE[0:8, COLIO_O + 1:COLIO_O + 1 + NB]
    wsc8 = megaQ[0:8, WSC_O:WSC_O + NB]
    for k in range(H - 1, 0, -1):
        v.scalar_tensor_tensor(
            out=wsc8, in0=iot8, scalar=megaQ[0:8, XF_O + k:XF_O + k + 1],
            in1=megaQ[0:8, GW_O + (k - 1) * NB:GW_O + k * NB],
            op0=AL.is_equal, op1=AL.mult,
            accum_out=megaQ[0:8, XF_O + k - 1:XF_O + k])
        if k % 32 == 1:
            b = k // 32
            # the last (b=0) scatter gates the olo/mask tail: use the
            # lower-latency SP HWDGE path for it instead of Pool SWDGE
            xq = nc.sync if b == 0 else nc.gpsimd
            for lane, hb in ((0, 0), (4, HBLK)):
                xq.dma_start(
                    out=megaRE[:, XC_O + hb + b:XC_O + hb + b + 1],
                    in_=megaQ[lane:lane + 4, XF_O + 32 * b:XF_O + 32 * b + 32])
    for bb in range(NBLK):
        v.tensor_tensor_scan(
            out=megaRE[:, R8_O + bb * CW + 1:R8_O + bb * CW + 1 + NB],
            data0=megaRE[:, R7_O + bb * CW + 1:R7_O + bb * CW + 1 + NB],
            data1=megaRE[:, R3_O + bb * CW + 1:R3_O + bb * CW + 1 + NB],
            initial=0.0, op0=AL.mult, op1=AL.add)        # Lfull -> R8

    # ---------------- olo + mask + metrics (bulk) ----------------
    # Sxy = sum(dcost * mask): inside the mask every cell is band-valid,
    # so dcost == |dx|+|dy| there (subcoef is ones per the input spec, so
    # Sx and Sy need not be separated).  BCE cells were precomputed in R1.
    xcolb = smb(XC_O)
    v.tensor_tensor(out=cells(R7_O), in0=ocolv(0), in1=xcolb, op=AL.is_equal)
    v.tensor_tensor(out=cells(R7_O), in0=cells(R7_O), in1=cells(R8_O),
                    op=AL.mult)
    v.tensor_reduce(out=megaRE[:, OLO_O:OLO_O + NBLK], in_=cells(R7_O),
                    axis=AX.X, op=AL.add)
    v.tensor_tensor(out=cells(R2_O), in0=ocolv(0), in1=smb(OLO_O),
                    op=AL.is_ge)
    v.tensor_tensor(out=cells(R7_O), in0=ocolv(0), in1=xcolb, op=AL.is_le)
    v.tensor_tensor(out=cells(R5_O), in0=cells(R2_O), in1=cells(R7_O),
                    op=AL.mult)                          # mask -> R5
    v.tensor_tensor(out=cells(R7_O), in0=cells(R6_O), in1=cells(R5_O),
                    op=AL.mult)
    v.tensor_reduce(out=megaRE[:, RED_O:RED_O + 1], in_=cells(R7_O),
                    axis=AX.XY, op=AL.add)
    v.tensor_tensor(out=cells(R2_O), in0=cells(R1_O), in1=cells(R5_O),
                    op=AL.mult)
    v.tensor_reduce(out=megaRE[:, RED_O + 2:RED_O + 3], in_=cells(R2_O),
                    axis=AX.XY, op=AL.add)
    # cnt closed form: per-row run length is x - lo + 1, so sum the tiny
    # [128,32] difference instead of reducing the full mask cells (host
    # adds the +1-per-row constant).
    v.tensor_tensor(out=megaRE[:, CLZ_O:CLZ_O + NBLK],
                    in0=megaRE[:, XC_O:XC_O + NBLK],
                    in1=megaRE[:, OLO_O:OLO_O + NBLK], op=AL.subtract)
    v.tensor_reduce(out=megaRE[:, RED_O + 3:RED_O + 4],
                    in_=megaRE[:, CLZ_O:CLZ_O + NBLK], axis=AX.X, op=AL.add)

    nc.sync.dma_start(out=partials[:], in_=megaRE[:, RED_O:RED_O + 4])


def _make_inmaps(preds, targs):
    preds = np.ascontiguousarray(preds, dtype=np.float32)
    targs = np.ascontiguousarray(targs, dtype=np.float32)
    pp = np.arange(32)
    bb = np.arange(NBLK)
    # row index per (p, b): F blocks b<16: i = 32b+p; R': i' = 32(b-16)+p
    iF = 32 * bb[None, :16] + pp[:, None]                # [32, 16]
    iR = 32 * (bb[None, 16:] - HBLK) + pp[:, None]
    idx = np.concatenate([iF, N - 1 - iR], axis=1)       # [32, 32] real rows

    uu = np.arange(SKW2)
    tF = uu[None, :] + pp[:, None] - 20                  # [32, SKW2]
    okF = (tF >= 0) & (tF < N)
    tFc = np.clip(tF, 0, N - 1)
    tR = 1043 - uu[None, :] - pp[:, None]
    okR = (tR >= 0) & (tR < N)
    tRc = np.clip(tR, 0, N - 1)

    cstrow = np.concatenate([np.arange(43),
                             np.arange(41, 0, -1)]).astype(np.float32)
    cstf = np.repeat(cstrow[None], 128, axis=0).copy()

    in_maps = []
    for c in range(NCORES):
        ps = preds[c * BC:(c + 1) * BC]                  # [4, N, F]
        ts = targs[c * BC:(c + 1) * BC]
        prev = np.zeros((4, 32, 3 * NBLK), np.float32)
        tskv = np.zeros((4, 32, 6 * SKW2), np.float32)
        for k in range(3):
            # poison out-of-range x/y targets so d is ~1e15 there (band
            # validity without an explicit mask); z stays 0 (masked out)
            pz = 0.0 if k == 2 else 5e14
            prev[:, :, k * NBLK:(k + 1) * NBLK] = ps[:, :, k][:, idx]
            tskv[:, :, k * SKW2:(k + 1) * SKW2] = \
                np.where(okF[None], ts[:, :, k][:, tFc], pz)
            tskv[:, :, (3 + k) * SKW2:(4 + k) * SKW2] = \
                np.where(okR[None], ts[:, :, k][:, tRc], pz)
        in_maps.append({"pre": prev.reshape(128, 3 * NBLK),
                        "tsk": tskv.reshape(128, 6 * SKW2), "cst": cstf})
    return in_maps


def _reduce_host(parts_list, subcoef):
    c0, c1 = float(subcoef[0]), float(subcoef[1])
    loss = 0.0
    for parts in parts_list:
        m = parts.reshape(BC, 32, 4).sum(axis=1)         # [s, (Sx,Sy,Sb,cnt)]
        for s in range(BC):
            sx, sy, sb, cnt = (float(m[s, k]) for k in range(4))
            loss += c0 * sx + c1 * sy + 0.1 * sb / (cnt + N)
    return np.float32(loss)


def _get_module():
    if "nc" not in _CACHE:
        _CACHE["nc"] = _build_module()
    return _CACHE["nc"]


def run(preds, targs, subcoef, trace=False):
    nc = _get_module()
    in_maps = _make_inmaps(preds, targs)
    res = run_bass_kernel_spmd(nc, in_maps, core_ids=list(range(NCORES)),
                               trace=trace)
    parts = [r["partials"] for r in res.results]
    return _reduce_host(parts, np.asarray(subcoef)), res


def kernel(preds, targs, subcoef):
    out, _ = run(preds, targs, subcoef)
    return out

